# revision 62
# baseline (speedup 1.0000x reference)
"""Multi-head attention kernel for Trainium2, 8 NeuronCores.

Problem: B=4, N=4096, C=256, H=4 heads, D=64.
  q,k,v = x@W{q,k,v}.T ; attn = softmax(q k^T / sqrt(D)) ; out = (attn v) @ Wp.T + bp

Sharding: 8 cores; core c handles batch b=c//2 and query-row half c%2
(2048 rows, all 4 heads). K/V cover the full 4096 sequence per batch, so
x[b]^T is passed whole to both of b's cores (columns rotated so that the
core's own query rows are always columns 0:2048 -- softmax over kv is
order-invariant as long as K and V share the order).

Engine balance (measured): ScalarE exp (33.5M exps/core, 1/1.2GHz/lane)
and the PE matmul stream (QK^T 109us + PV 109us + projections ~35us,
out-free-size-bound at 2.4GHz) are both ~253us busy, so the schedule is
a two-engine makespan problem; the TimelineSim cost model reproduces HW
within a few us and was used to place work.

Per-core pipeline (engine picks all explicit):
  - projections on PE (bf16): K^T/Q^T in [128 (d duplicated twice), n]
    layout; the duplication lets consecutive kv-chunk QK^T matmuls
    alternate between the two 64-row PE quadrants so each chunk's
    weight load hides under the previous chunk's moving stream
    (A/B-measured: removing alternation costs ~90us, interleaving
    projections into the attention stream costs ~25us on HW even though
    the sim predicts a win -- full-array projection weight loads break
    the alternation pipelining; so all projections run up front).
    V in [kv, d] layout with 64 ones-columns appended (the PV matmul
    then emits the softmax denominator replicated on partitions 64..127
    for free -- output partitions are free, moving free size is what
    costs). Startup: input DMAs are ordered so the first projection's
    operands (wq, leading 512 xT columns) land first, and the v-ones /
    o_sb memsets run on the idle Pool engine so DVE can copy projection
    PSUM out immediately.
  - attention units (h, qt): S^T = K Q^T into PSUM ([128 kv, 512 rows]
    per kv chunk, 3 chunks per PSUM tile = the bank budget; spool 6 +
    apool 2 banks), exp on ScalarE ([128,1536] blocks, scale=1/sqrt(D)
    fused) -> P^T bf16 in SBUF. Each unit's PV accumulation runs inside
    the NEXT unit (software pipeline, paced by a PE token bucket per exp
    group) so it overlaps that unit's exp stream. fp8 was measured and
    rejected: ACT fp8 output costs +32us, and DoubleRow/SwInterleave
    matmuls are slower than bf16 on this HW despite the 0.5 cycles/row
    cost model.
  - normalize: [128, 512] PSUM acc has O'^T on partitions 0:64 and the
    softmax sums replicated on 64:128; DVE reciprocal there, SBUF->SBUF
    DMA shifts it to partitions 0:64, DVE multiply -> O^T bf16.
  - output projection per qtile right after its last head (4
    accumulating matmuls, contraction = each head's 64 dims) +
    host-prebroadcast bias added on the PSUM->SBUF copy; the four
    128-row blocks accumulate in one SBUF staging tile and ship as a
    single strided DMA (HWDGE descriptor generation, 625ns each, is the
    tail serializer).
"""

import os
import numpy as np
import ml_dtypes

B, N, C, H, D = 4, 4096, 256, 4, 64
SCALE = float(D) ** -0.5
NCORES = 8
RPC = N // 2          # query rows per core
QT = 512              # rows per query tile
NQT = RPC // QT       # 4
KVC = 128             # kv chunk (PSUM partition dim)
NKV = N // KVC        # 32
GRP = 3               # kv chunks per exp block ([128, 1536] = 3 PSUM banks)
GROUP_NS = 1150       # PE lag-work budget granted per exp group (token bucket)
BURST_NS = 2400       # budget carryover cap (limits PE burst per group)
KQW = 900             # queue weight: one K/Q projection tile (2 matmuls)
VPW = 900             # queue weight: one V pair (4 small matmuls)
PVW = 1000            # queue weight: one PV burst (4 chunks, bf16)
NORM_PE = False       # PE eye-matmul shift: nondeterministic tile-scheduler
                      # deadlocks (extra PSUM tile in the acc rotation); keep
                      # the DMA partition-shift instead
WARMUP = 0            # junk matmuls at t=0 to ramp the PE p-state while the
                      # input DMAs land (0 = off)

_cache = {}


def _build_program(num_units=None, repeat=1, mode="upfront", pv="dri"):
    import concourse.bacc as bacc
    import concourse.mybir as mybir
    import concourse.tile as tile
    from contextlib import ExitStack

    BF16 = mybir.dt.bfloat16
    F32 = mybir.dt.float32
    F8 = mybir.dt.float8e4
    MPM = mybir.MatmulPerfMode
    AF = mybir.ActivationFunctionType

    nc = bacc.Bacc()

    xT = nc.declare_dram_parameter("xT", [2, 128, N], BF16, False)
    eye = nc.declare_dram_parameter("eye", [128, 64], BF16, False)
    wq = nc.declare_dram_parameter("wq", [2, 128, H * 128], BF16, False)
    wk = nc.declare_dram_parameter("wk", [2, 128, H * 128], BF16, False)
    wv = nc.declare_dram_parameter("wv", [2, 128, C], BF16, False)
    wp = nc.declare_dram_parameter("wp", [H, 64, C], BF16, False)
    bp = nc.declare_dram_parameter("bp", [128, C], F32, False)  # pre-broadcast
    out = nc.declare_dram_parameter("out", [RPC, C], F32, True)

    with ExitStack() as ctx:
        tc = ctx.enter_context(tile.TileContext(nc))
        cpool = ctx.enter_context(tc.tile_pool(name="consts", bufs=1))
        spool = ctx.enter_context(tc.tile_pool(name="spsum", bufs=2, space="PSUM"))
        apool = ctx.enter_context(tc.tile_pool(name="apsum", bufs=2, space="PSUM"))
        ppool = ctx.enter_context(tc.tile_pool(name="ptile", bufs=2))
        npool = ctx.enter_context(tc.tile_pool(name="norm", bufs=3))
        opool = ctx.enter_context(tc.tile_pool(name="outs", bufs=1))
        if repeat > 1:
            ctx.enter_context(tc.For_i(0, repeat, 1))

        # ---- load inputs ----
        # Order minimizes time-to-first-projection: wq/wk first (small),
        # then the leading 512 columns of xT (all head-0 j=0 projections
        # need), then the xT bulk and the later-phase weights.
        xT_sb = [cpool.tile([128, N], BF16, name=f"xT{cc}") for cc in range(2)]
        wq_sb = [cpool.tile([128, H * 128], BF16, name=f"wq{cc}")
                 for cc in range(2)]
        wk_sb = [cpool.tile([128, H * 128], BF16, name=f"wk{cc}")
                 for cc in range(2)]
        wv_sb = [cpool.tile([128, C], BF16, name=f"wv{cc}") for cc in range(2)]
        # SP queue: what the first projections need; ACT queue (idle at
        # startup): the xT bulk + wv, in parallel with SP's descriptors
        for cc in range(2):
            nc.sync.dma_start(wq_sb[cc][:], wq[cc])
            nc.sync.dma_start(wk_sb[cc][:], wk[cc])
        for cc in range(2):
            nc.sync.dma_start(xT_sb[cc][:, 0:512], xT[cc, :, 0:512])
        for cc in range(2):
            nc.sync.dma_start(xT_sb[cc][:, 512:N // 2], xT[cc, :, 512:N // 2])
            nc.sync.dma_start(xT_sb[cc][:, N // 2:N], xT[cc, :, N // 2:N])
        for cc in range(2):
            nc.sync.dma_start(wv_sb[cc][:], wv[cc])
        wp_sb = []
        for h in range(H):
            t = cpool.tile([64, C], BF16, name=f"wp{h}")
            nc.sync.dma_start(t[:], wp[h])
            wp_sb.append(t)
        bp_sb = cpool.tile([128, C], F32, name="bp_sb")
        nc.sync.dma_start(bp_sb[:], bp[:])
        # identity block on partitions 64:128: PE-shifts the softmax-sum
        # reciprocals from partitions 64:128 down to 0:64 (no DMA round-trip)
        eye_sb = cpool.tile([128, 64], BF16, name="eye_sb")
        nc.sync.dma_start(eye_sb[:], eye[:])

        if WARMUP:
            # ramp the PE p-state (full speed needs ~3us of continuous
            # execution) on junk data while the input DMAs land, so the
            # first projections run at 2.4GHz
            junk = cpool.tile([64, 64], BF16, name="junk")
            nc.gpsimd.memset(junk[:], 0.5)
            jp = apool.tile([64, 64], F32, tag="acc", name="jp")
            for _ in range(WARMUP):
                nc.tensor.matmul(jp[:], junk[:], junk[:],
                                 start=True, stop=True)

        # ---- persistent SBUF tensors ----
        kt_sb = [cpool.tile([128, N], BF16, name=f"kt{h}") for h in range(H)]
        qt_sb = [cpool.tile([128, RPC], BF16, name=f"qt{h}") for h in range(H)]
        PVDT = BF16 if pv == "bf16" else F8
        v_sb = cpool.tile([128, NKV, H, 128], PVDT, name="v_sb")
        # ones columns for the PV denominator trick: memset on the (idle)
        # Pool engine so DVE is free for projection copies at startup
        nc.gpsimd.memset(v_sb[:, :, :, 64:128], 1.0)
        o_sb = [cpool.tile([64, RPC], BF16, name=f"o{h}") for h in range(H)]
        micro = mode in ("qkonly", "pvonly")
        nu_ = H * NQT
        if micro or mode == "nopv" or (
                num_units is not None and num_units < nu_):
            # partial-unit timing variants read o_sb regions no unit wrote
            for h in range(H):
                nc.gpsimd.memset(o_sb[h][:], 0.0)
        if micro:
            for h in range(H):
                nc.vector.memset(kt_sb[h][:], 0.125)
                nc.vector.memset(qt_sb[h][:], 0.125)
            nc.vector.memset(v_sb[:], 0.125)
        pt_shared = None
        if mode == "pvonly":
            pt_shared = cpool.tile([128, NKV * QT], PVDT, name="pt_shared")
            nc.vector.memset(pt_shared[:], 0.125)

        def kq_tile(h, kind, j, tag="acc", pool=None):
            """One K^T or Q^T projection tile (512 cols) for head h."""
            w_sb, dst = (wk_sb, kt_sb) if kind == "k" else (wq_sb, qt_sb)
            ps = (pool or apool).tile([128, 512], F32, tag=tag, name=f"{kind}proj")
            for cc in range(2):
                nc.tensor.matmul(
                    ps[:],
                    w_sb[cc][:, h * 128:(h + 1) * 128],
                    xT_sb[cc][:, j * 512:(j + 1) * 512],
                    start=(cc == 0), stop=(cc == 1),
                )
            nc.vector.tensor_copy(dst[h][:, j * 512:(j + 1) * 512], ps[:])

        def kq_proj_work(h):
            return [
                (900, (lambda kind=kind, j=j: kq_tile(h, kind, j)))
                for kind, nj in (("k", N // 512), ("q", RPC // 512))
                for j in range(nj)
            ]

        def v_pair(j2):
            """V projection for kv chunks 2*j2, 2*j2+1 (one PSUM tile)."""
            ps = apool.tile([128, 512], F32, tag="acc", name="vproj")
            for u in range(2):
                j = 2 * j2 + u
                for cc in range(2):
                    nc.tensor.matmul(
                        ps[:, u * C:(u + 1) * C],
                        xT_sb[cc][:, j * 128:(j + 1) * 128],
                        wv_sb[cc][:],
                        start=(cc == 0), stop=(cc == 1),
                    )
            nc.vector.tensor_copy(
                v_sb[:, 2 * j2:2 * j2 + 2, :, 0:64],
                ps[:].rearrange("p (u h d) -> p u h d", u=2, h=H),
            )

        groups = []
        j = 0
        while j < NKV:
            groups.append((j, min(NKV, j + GRP)))
            j += GRP
        NGRP = len(groups)

        ot_state = {}

        def rowtile_final(qt, rt, flush=None):
            # accumulate the qtile's 128-row output blocks in one SBUF
            # staging tile; strided DMAs ship multi-block spans (HWDGE
            # descriptor generation is the tail serializer, so fewer DMAs)
            if flush is None:
                flush = rt == QT // 128 - 1
            r0 = qt * QT + rt * 128
            po = apool.tile([128, C], F32, tag="acc", name="po")
            for h in range(H):
                nc.tensor.matmul(
                    po[:],
                    o_sb[h][:, r0:r0 + 128],
                    wp_sb[h][:],
                    start=(h == 0), stop=(h == H - 1),
                )
            if qt not in ot_state:
                ot_state[qt] = [
                    opool.tile([128, QT // 128, C], F32, tag="ot", name="ot"),
                    0,
                ]
            tile, fs = ot_state[qt]
            nc.vector.tensor_add(tile[:, rt, :], po[:], bp_sb[:])
            if flush:
                nt = rt - fs + 1
                nc.sync.dma_start(
                    out[qt * QT + fs * 128:qt * QT + (rt + 1) * 128, :]
                    .rearrange("(t p) c -> p t c", t=nt),
                    tile[:, fs:rt + 1, :],
                )
                ot_state[qt][1] = rt + 1

        def pv_norm_work(h, qt, pt, sub=False):
            """Closures for unit (h, qt)'s PV burst (4 chunks each), then
            normalize, then (after head 3) the qtile's output projection.
            sub=True (last unit) pipelines normalize + output projection in
            128-row subtiles so the post-exp tail is short. The PSUM acc
            tile is allocated lazily at the first pv4 call so queued work
            from other units can rotate through the pool meanwhile."""
            cell = {}

            def getacc():
                if "a" not in cell:
                    cell["a"] = apool.tile([128, QT], F32, tag="acc",
                                           name="acc")
                return cell["a"]

            def pv4(j0):
                acc = getacc()
                if pv in ("dr", "dri"):
                    # fp8 DoubleRow: two kv chunks (256-deep contraction)
                    # per matmul; pt chunk pair [128, 2*QT] is pair-major.
                    pm = MPM.DoubleRow if pv == "dr" else \
                        MPM.DoubleRowSwInterleave
                    for j2 in range(j0 // 2, j0 // 2 + 2):
                        nc.tensor.matmul(
                            acc[:],
                            v_sb[:, 2 * j2:2 * j2 + 2, h, :],
                            pt[:, 2 * j2 * QT:(2 * j2 + 2) * QT].rearrange(
                                "p (t r) -> p t r", t=2),
                            start=(j2 == 0), stop=(j2 == NKV // 2 - 1),
                            perf_mode=pm,
                        )
                else:
                    for jj in range(j0, j0 + 4):
                        nc.tensor.matmul(
                            acc[:],
                            v_sb[:, jj, h, :],
                            pt[:, jj * QT:jj * QT + QT],
                            start=(jj == 0), stop=(jj == NKV - 1),
                        )

            def norm_noop():
                pass

            def norm(c0=0, w=QT, direct=False):
                # partitions 0:64 = O'^T, 64:128 = sums (replicated).
                acc = getacc()
                if NORM_PE:
                    # Reciprocal on DVE (bf16 out), PE eye-matmul shifts it
                    # to partitions 0:64 in PSUM, then one DVE multiply
                    # reads both PSUM operands and writes o_sb.
                    rcp = npool.tile([128, w], BF16, tag="rcp", name="rcp")
                    with nc.allow_low_precision(
                            reason="softmax reciprocal in bf16 feeds a bf16 "
                                   "multiply; 2e-2 rel-err budget"):
                        nc.vector.reciprocal(rcp[64:128, :],
                                             acc[64:128, c0:c0 + w])
                    sh = apool.tile([64, w], F32, tag="acc", name="sh")
                    nc.tensor.matmul(
                        sh[:], eye_sb[64:128, :], rcp[64:128, :],
                        start=True, stop=True, tile_position=(64, 0),
                    )
                    nc.vector.tensor_mul(
                        o_sb[h][:, qt * QT + c0:qt * QT + c0 + w],
                        acc[0:64, c0:c0 + w], sh[:],
                    )
                    return
                rcp = npool.tile([128, w], F32, tag="rcp", name="rcp")
                nc.vector.reciprocal(rcp[64:128, :], acc[64:128, c0:c0 + w])
                bc = npool.tile([64, w], F32, tag="bc", name="bc")
                nc.sync.dma_start(bc[:], rcp[64:128, :])
                if direct:
                    # tail path: multiply straight out of PSUM (no copy);
                    # holding the acc slot a little longer is free here
                    nc.vector.tensor_mul(
                        o_sb[h][:, qt * QT + c0:qt * QT + c0 + w],
                        acc[0:64, c0:c0 + w], bc[:],
                    )
                    return
                onum = npool.tile([64, w], F32, tag="onum", name="onum")
                nc.vector.tensor_copy(onum[:], acc[0:64, c0:c0 + w])
                nc.vector.tensor_mul(
                    o_sb[h][:, qt * QT + c0:qt * QT + c0 + w], onum[:], bc[:],
                )

            if mode in ("nopv", "qkonly"):
                work = []
            else:
                pvw = 250 if pv in ("dr", "dri") else PVW
                work = [(pvw, (lambda j0=j0: pv4(j0))) for j0 in range(0, NKV, 4)]
                if sub and not micro and h == H - 1:
                    # last unit: pipeline normalize + output projection in
                    # two 256-row halves so the post-exp tail is short; both
                    # norm DMAs issue before the first output projection
                    hw_ = QT // 2
                    work.append((150, lambda: norm(0, hw_, True)))
                    work.append((150, lambda: norm(hw_, hw_, True)))
                    work.append((600, lambda: rowtile_final(qt, 0, False)))
                    work.append((600, lambda: rowtile_final(qt, 1, True)))
                    work.append((600, lambda: rowtile_final(qt, 2, False)))
                    work.append((600, lambda: rowtile_final(qt, 3, True)))
                    return work
                work.append((150, norm_noop if micro else norm))
            if h == H - 1:
                work.extend(
                    (600, (lambda rt=rt: rowtile_final(qt, rt)))
                    for rt in range(QT // 128)
                )
            return work

        def emit_unit_fifo(h, qt, queue, bcell, drain):
            """QK^T + exp stream for unit (h, qt) under the shared FIFO work
            queue: after each exp group, this unit's own PV entries whose pt
            chunks are now produced are appended, then queued closures run
            under the PE-slack token bucket. No drain at unit end (backlog
            carries across units) unless drain=True (last unit)."""
            pt = ppool.tile([128, NKV * QT], PVDT, tag="pt", name="pt")
            own = list(pv_norm_work(h, qt, pt, sub=drain))
            npv = NKV // 4 if own else 0
            for gi, (j0, j1) in enumerate(groups):
                w = (j1 - j0) * QT
                st = spool.tile([128, GRP * QT], F32, tag="st", name="st")
                for jj in range(j0, j1):
                    rg = (jj % 2) * 64
                    nc.tensor.matmul(
                        st[:, (jj - j0) * QT:(jj - j0) * QT + QT],
                        kt_sb[h][rg:rg + 64, jj * KVC:(jj + 1) * KVC],
                        qt_sb[h][rg:rg + 64, qt * QT:qt * QT + QT],
                        start=True, stop=True,
                        tile_position=(rg, 0),
                    )
                if mode == "noact":
                    nc.scalar.activation(
                        pt[:, j0 * QT:j0 * QT + 64], st[:, 0:64], AF.Exp,
                        scale=SCALE,
                    )
                else:
                    nc.scalar.activation(
                        pt[:, j0 * QT:j1 * QT], st[:, 0:w], AF.Exp, scale=SCALE,
                    )
                # this unit's pv4(j0=4k) is runnable once chunks j0..j0+3
                # are exp'd (group floor((j0+3)/GRP)); the trailing
                # norm/output closures follow the last pv4 in FIFO order
                while npv and 4 * (NKV // 4 - npv) + 3 <= GRP * gi + 2:
                    queue.append(own.pop(0))
                    npv -= 1
                    if npv == 0:
                        queue.extend(own)
                        del own[:]
                budget = min(bcell[0], BURST_NS) + GROUP_NS
                while queue and queue[0][0] <= budget:
                    c, fn = queue.pop(0)
                    fn()
                    budget -= c
                bcell[0] = budget
            if own:
                queue.extend(own)
                del own[:]
            if drain:
                for c, fn in queue:
                    fn()
                del queue[:]
            return pt

        def emit_unit(h, qt, pvq, extra, st_extra=None, self_pv=False,
                      own_push=None):
            """QK^T + exp stream for unit (h, qt); interleaves the previous
            unit's PV/norm closures (pvq), projection closures (extra: [acc]
            slots; st_extra: [st] slots), and optionally its own PV at a
            3-group lag (last unit). own_push (fifo mode): callback(gi) that
            appends this unit's own PV work to the shared queue as the exp
            groups that produce its inputs retire."""
            if mode == "pvonly":
                for c, fn in pvq:
                    fn()
                del pvq[:]
                return pt_shared
            pt = ppool.tile([128, NKV * QT], PVDT, tag="pt", name="pt")
            own = pv_norm_work(h, qt, pt) if self_pv else None
            budget = 0
            for gi, (j0, j1) in enumerate(groups):
                w = (j1 - j0) * QT
                st = spool.tile([128, GRP * QT], F32, tag="st", name="st")
                qw = 64 if mode == "smallqk" else QT
                for jj in range(j0, j1):
                    rg = 0 if mode == "qkflat" else (jj % 2) * 64
                    nc.tensor.matmul(
                        st[:, (jj - j0) * QT:(jj - j0) * QT + qw],
                        kt_sb[h][rg:rg + 64, jj * KVC:(jj + 1) * KVC],
                        qt_sb[h][rg:rg + 64, qt * QT:qt * QT + qw],
                        start=True, stop=True,
                        tile_position=None if mode == "qkflat" else (rg, 0),
                    )
                if mode == "qkonly":
                    pass
                elif mode == "noact":
                    nc.scalar.activation(
                        pt[:, j0 * QT:j0 * QT + 64], st[:, 0:64], AF.Exp,
                        scale=SCALE,
                    )
                else:
                    nc.scalar.activation(
                        pt[:, j0 * QT:j1 * QT], st[:, 0:w], AF.Exp, scale=SCALE,
                    )
                if st_extra:
                    st_extra.pop(0)()
                if own_push is not None:
                    own_push(gi)
                budget = min(budget, BURST_NS) + GROUP_NS
                while pvq and pvq[0][0] <= budget:
                    c, fn = pvq.pop(0)
                    fn()
                    budget -= c
                if own and gi >= 3:
                    own.pop(0)[1]()
            for c, fn in pvq:
                fn()
            del pvq[:]
            if own:
                for c, fn in own:
                    fn()
            return pt

        # ---- pipeline ----
        # Only the first K and Q projection tiles of head 0 run up front (the
        # minimum for unit 0's first QK^T group); the rest of head 0's tiles
        # ride unit 0's [st] rotation (each tile j lands before the group
        # that needs it). V is interleaved into units 0-1 through the [acc]
        # slots (free there since each unit's PV runs inside the NEXT unit);
        # head h+1's K/Q projections spread over head h's units.
        if not micro:
            if mode in ("fifo", "noact"):
                kq_tile(0, "q", 0)
                kq_tile(0, "k", 0)
            else:
                kq_tile(0, "q", 0, tag="st", pool=spool)
                kq_tile(0, "k", 0, tag="st", pool=spool)
        st0 = [] if micro else [
            (lambda j=j: kq_tile(0, "k", j, tag="st", pool=spool))
            for j in range(1, N // 512)
        ] + [
            (lambda j=j: kq_tile(0, "q", j, tag="st", pool=spool))
            for j in range(1, RPC // 512)
        ]

        units = [(h, qt) for h in range(H) for qt in range(NQT)]
        extras = {i: [] for i in range(len(units))}
        vps = [] if micro else [
            (900, (lambda j2=j2: v_pair(j2))) for j2 in range(NKV // 2)
        ]
        extras[0].extend(vps[:8])
        extras[1].extend(vps[8:])
        for nh in range(H if micro else 1, H):
            w = kq_proj_work(nh)
            for k in range(NQT):
                lo = (len(w) * k) // NQT
                hi = (len(w) * (k + 1)) // NQT
                extras[(nh - 1) * NQT + k].extend(w[lo:hi])

        def interleave(a, b):
            out, la, lb = [], list(a), list(b)
            while la or lb:
                if la:
                    out.append(la.pop(0))
                if lb:
                    out.append(lb.pop(0))
            return out

        if mode in ("fifo", "noact") and not micro:
            # single shared FIFO: head-0 K/Q tiles first (feeding unit 0's
            # own groups just-in-time), then V pairs alternating with
            # heads 1-3 K/Q projections; each unit's PV/norm/output work is
            # appended by emit_unit_fifo as its inputs appear. Pops happen
            # after each exp group under the PE-slack token bucket, so the
            # exp stream is never displaced by more than BURST_NS of PE work.
            queue = []
            for j in range(1, N // 512):
                queue.append((KQW, (lambda j=j: kq_tile(0, "k", j))))
            for j in range(1, RPC // 512):
                queue.append((KQW, (lambda j=j: kq_tile(0, "q", j))))
            rest = []
            for nh in range(1, H):
                rest.extend(
                    (KQW, fn2) for _c, fn2 in kq_proj_work(nh))
            vq = [(VPW, (lambda j2=j2: v_pair(j2)))
                  for j2 in range(NKV // 2)]
            queue.extend(interleave(vq, rest))
            nu = len(units) if num_units is None else num_units
            bcell = [0]
            for i, (h, qt) in enumerate(units[:nu]):
                emit_unit_fifo(h, qt, queue, bcell, drain=(i == nu - 1))
            if nu < len(units):
                for c, fn in queue:
                    fn()
                for qt in range(NQT):
                    for rt in range(QT // 128):
                        rowtile_final(qt, rt)
            return nc

        if mode in ("upfront", "mid"):
            if mode == "upfront":
                for w in st0:
                    w()
                del st0[:]
            for i in range(len(units)):
                for c, fn in extras[i]:
                    fn()
                extras[i] = []

        nu = len(units) if num_units is None else num_units
        pvq = []
        for i, (h, qt) in enumerate(units[:nu]):
            last = i == nu - 1
            if i == 1:
                # unit 1's leftover V pairs must precede unit 0's PV chunks
                q = interleave(extras[i][:8], pvq) + extras[i][8:]
            else:
                q = interleave(pvq, extras[i]) if extras[i] else pvq
            pt = emit_unit(
                h, qt, q, [],
                st_extra=(st0 if i == 0 else None), self_pv=last,
            )
            pvq = [] if last else pv_norm_work(h, qt, pt)
        for c, fn in pvq:
            fn()
        if nu < len(units):
            # timing variants: keep PE/DVE/DMA side work + full output
            # traffic identical, just skip the later attention units
            for i in range(nu, len(units)):
                for c, fn in extras[i]:
                    fn()
            for qt in range(NQT):
                for rt in range(QT // 128):
                    rowtile_final(qt, rt)

    return nc


PV_DEFAULT = "bf16"


def _get_program(num_units=None, repeat=1, mode="upfront", pv=None):
    if pv is None:
        pv = PV_DEFAULT
    key = ("nc", num_units, repeat, mode, pv)
    if key not in _cache:
        nc = _build_program(num_units, repeat, mode, pv)
        if not nc.is_finalized():
            nc.finalize()
        _cache[key] = nc
    return _cache[key]


def _prep_inputs(x, Wq, Wk, Wv, Wp, bpv):
    """Build the 8 per-core input maps (host-side shard prep)."""
    bf16 = ml_dtypes.bfloat16
    x = np.asarray(x, dtype=np.float32)

    def dup_heads(w):
        # W [out=C, in=C] (torch Linear) -> W.T [C_in, C_out]; per head 64
        # output cols duplicated to 128; chunk contraction dim into 2x128.
        wt = np.ascontiguousarray(np.asarray(w, dtype=np.float32).T)  # [C_in, C_out]
        cols = []
        for h in range(H):
            blk = wt[:, h * 64:(h + 1) * 64]
            cols.append(np.concatenate([blk, blk], axis=1))  # [C, 128]
        arr = np.concatenate(cols, axis=1)                   # [C, H*128]
        return np.ascontiguousarray(arr.reshape(2, 128, H * 128)).astype(bf16)

    wq_a = dup_heads(Wq)
    wk_a = dup_heads(Wk)
    wv_a = np.ascontiguousarray(
        np.asarray(Wv, dtype=np.float32).T.reshape(2, 128, C)
    ).astype(bf16)
    wp_a = np.ascontiguousarray(
        np.asarray(Wp, dtype=np.float32).T.reshape(H, 64, C)
    ).astype(bf16)
    bp_a = np.ascontiguousarray(
        np.broadcast_to(np.asarray(bpv, dtype=np.float32).reshape(1, C), (128, C))
    )

    eye_a = np.zeros((128, 64), dtype=np.float32)
    eye_a[64:128, :] = np.eye(64, dtype=np.float32)
    eye_a = eye_a.astype(bf16)

    in_maps = []
    for c in range(NCORES):
        b, half = c // 2, c % 2
        xt = np.ascontiguousarray(x[b].T)  # [C, N]
        if half == 1:
            xt = np.concatenate([xt[:, RPC:], xt[:, :RPC]], axis=1)
        in_maps.append({
            "xT": np.ascontiguousarray(xt.reshape(2, 128, N)).astype(bf16),
            "wq": wq_a, "wk": wk_a, "wv": wv_a, "wp": wp_a, "bp": bp_a,
            "eye": eye_a,
        })
    return in_maps


def run_cores(in_maps, trace=False):
    from concourse.bass_utils import run_bass_kernel_spmd
    nc = _get_program()
    return run_bass_kernel_spmd(
        nc, in_maps, list(range(NCORES)), trace=trace,
    )


def kernel(x, Wq, Wk, Wv, Wp, bp):
    in_maps = _prep_inputs(x, Wq, Wk, Wv, Wp, bp)
    res = run_cores(in_maps, trace=bool(int(os.environ.get("KERNEL_TRACE", "0"))))
    full = np.empty((B, N, C), dtype=np.float32)
    for c in range(NCORES):
        b, half = c // 2, c % 2
        full[b, half * RPC:(half + 1) * RPC, :] = res.results[c]["out"]
    _cache["last_results"] = res
    return full



# revision 63
# speedup vs baseline: 1.1595x; 1.1595x over previous
"""Multi-head attention kernel for Trainium2, 8 NeuronCores.

Problem: B=4, N=4096, C=256, H=4 heads, D=64.
  q,k,v = x@W{q,k,v}.T ; attn = softmax(q k^T / sqrt(D)) ; out = (attn v) @ Wp.T + bp

Sharding: 8 cores; core c handles batch b=c//2 and query-row half c%2
(2048 rows, all 4 heads). K/V cover the full 4096 sequence per batch, so
x[b]^T is passed whole to both of b's cores (columns rotated so that the
core's own query rows are always columns 0:2048 -- softmax over kv is
order-invariant as long as K and V share the order).

Engine balance (measured): ScalarE exp (33.5M exps/core, 1/1.2GHz/lane)
and the PE matmul stream (QK^T 109us + PV 109us + projections ~35us,
out-free-size-bound at 2.4GHz) are both ~253us busy, so the schedule is
a two-engine makespan problem; the TimelineSim cost model reproduces HW
within a few us and was used to place work.

Per-core pipeline (engine picks all explicit):
  - projections on PE (bf16): K^T/Q^T in [128 (d duplicated twice), n]
    layout; the duplication lets consecutive kv-chunk QK^T matmuls
    alternate between the two 64-row PE quadrants so each chunk's
    weight load hides under the previous chunk's moving stream
    (A/B-measured: removing alternation costs ~90us, interleaving
    projections into the attention stream costs ~25us on HW even though
    the sim predicts a win -- full-array projection weight loads break
    the alternation pipelining; so all projections run up front).
    V in [kv, d] layout with 64 ones-columns appended (the PV matmul
    then emits the softmax denominator replicated on partitions 64..127
    for free -- output partitions are free, moving free size is what
    costs). Startup: input DMAs are ordered so the first projection's
    operands (wq, leading 512 xT columns) land first, and the v-ones /
    o_sb memsets run on the idle Pool engine so DVE can copy projection
    PSUM out immediately.
  - attention units (h, qt): S^T = K Q^T into PSUM ([128 kv, 512 rows]
    per kv chunk, 3 chunks per PSUM tile = the bank budget; spool 6 +
    apool 2 banks), exp on ScalarE ([128,1536] blocks, scale=1/sqrt(D)
    fused) -> P^T bf16 in SBUF. Each unit's PV accumulation runs inside
    the NEXT unit (software pipeline, paced by a PE token bucket per exp
    group) so it overlaps that unit's exp stream. fp8 was measured and
    rejected: ACT fp8 output costs +32us, and DoubleRow/SwInterleave
    matmuls are slower than bf16 on this HW despite the 0.5 cycles/row
    cost model.
  - normalize: [128, 512] PSUM acc has O'^T on partitions 0:64 and the
    softmax sums replicated on 64:128; DVE reciprocal there, SBUF->SBUF
    DMA shifts it to partitions 0:64, DVE multiply -> O^T bf16.
  - output projection per qtile right after its last head (4
    accumulating matmuls, contraction = each head's 64 dims) +
    host-prebroadcast bias added on the PSUM->SBUF copy; the four
    128-row blocks accumulate in one SBUF staging tile and ship as a
    single strided DMA (HWDGE descriptor generation, 625ns each, is the
    tail serializer).
"""

import os
import numpy as np
import ml_dtypes

B, N, C, H, D = 4, 4096, 256, 4, 64
SCALE = float(D) ** -0.5
NCORES = 8
RPC = N // 2          # query rows per core
QT = 512              # rows per query tile
NQT = RPC // QT       # 4
KVC = 128             # kv chunk (PSUM partition dim)
NKV = N // KVC        # 32
GRP = 3               # kv chunks per exp block ([128, 1536] = 3 PSUM banks)
GROUP_NS = 1150       # PE lag-work budget granted per exp group (token bucket)
BURST_NS = 2400       # budget carryover cap (limits PE burst per group)
KQW = 900             # queue weight: one K/Q projection tile (2 matmuls)
VPW = 900             # queue weight: one V pair (4 small matmuls)
PVW = 1000            # queue weight: one PV burst (4 chunks, bf16)
NORM_PE = False       # PE eye-matmul shift: nondeterministic tile-scheduler
                      # deadlocks (extra PSUM tile in the acc rotation); keep
                      # the DMA partition-shift instead
WARMUP = 0            # junk matmuls at t=0 to ramp the PE p-state while the
                      # input DMAs land (0 = off)

_cache = {}


def _build_program(num_units=None, repeat=1, mode="upfront", pv="bf16"):
    import concourse.bacc as bacc
    import concourse.mybir as mybir
    import concourse.tile as tile
    from contextlib import ExitStack

    BF16 = mybir.dt.bfloat16
    F32 = mybir.dt.float32
    F8 = mybir.dt.float8e4
    MPM = mybir.MatmulPerfMode
    AF = mybir.ActivationFunctionType

    nc = bacc.Bacc()

    xT = nc.declare_dram_parameter("xT", [2, 128, N], BF16, False)
    eye = nc.declare_dram_parameter("eye", [128, 64], BF16, False)
    wq = nc.declare_dram_parameter("wq", [2, 128, H * 128], BF16, False)
    wk = nc.declare_dram_parameter("wk", [2, 128, H * 128], BF16, False)
    wv = nc.declare_dram_parameter("wv", [2, 128, C], BF16, False)
    wp = nc.declare_dram_parameter("wp", [H, 64, C], BF16, False)
    bp = nc.declare_dram_parameter("bp", [128, C], F32, False)  # pre-broadcast
    out = nc.declare_dram_parameter("out", [RPC, C], F32, True)

    with ExitStack() as ctx:
        tc = ctx.enter_context(tile.TileContext(nc))
        cpool = ctx.enter_context(tc.tile_pool(name="consts", bufs=1))
        spool = ctx.enter_context(tc.tile_pool(name="spsum", bufs=2, space="PSUM"))
        apool = ctx.enter_context(tc.tile_pool(name="apsum", bufs=2, space="PSUM"))
        ppool = ctx.enter_context(tc.tile_pool(name="ptile", bufs=2))
        npool = ctx.enter_context(tc.tile_pool(name="norm", bufs=3))
        opool = ctx.enter_context(tc.tile_pool(name="outs", bufs=1))
        if repeat > 1:
            ctx.enter_context(tc.For_i(0, repeat, 1))

        # ---- load inputs ----
        # Order minimizes time-to-first-projection: wq/wk first (small),
        # then the leading 512 columns of xT (all head-0 j=0 projections
        # need), then the xT bulk and the later-phase weights.
        xT_sb = [cpool.tile([128, N], BF16, name=f"xT{cc}") for cc in range(2)]
        wq_sb = [cpool.tile([128, H * 128], BF16, name=f"wq{cc}")
                 for cc in range(2)]
        wk_sb = [cpool.tile([128, H * 128], BF16, name=f"wk{cc}")
                 for cc in range(2)]
        wv_sb = [cpool.tile([128, C], BF16, name=f"wv{cc}") for cc in range(2)]
        # SP queue: what the first projections need; ACT queue (idle at
        # startup): the xT bulk + wv, in parallel with SP's descriptors
        for cc in range(2):
            nc.sync.dma_start(wq_sb[cc][:], wq[cc])
            nc.sync.dma_start(wk_sb[cc][:], wk[cc])
        for cc in range(2):
            nc.sync.dma_start(xT_sb[cc][:, 0:512], xT[cc, :, 0:512])
        for cc in range(2):
            nc.sync.dma_start(xT_sb[cc][:, 512:N // 2], xT[cc, :, 512:N // 2])
            nc.sync.dma_start(xT_sb[cc][:, N // 2:N], xT[cc, :, N // 2:N])
        for cc in range(2):
            nc.sync.dma_start(wv_sb[cc][:], wv[cc])
        wp_sb = []
        for h in range(H):
            t = cpool.tile([64, C], BF16, name=f"wp{h}")
            nc.sync.dma_start(t[:], wp[h])
            wp_sb.append(t)
        bp_sb = cpool.tile([128, C], F32, name="bp_sb")
        nc.sync.dma_start(bp_sb[:], bp[:])
        # identity block on partitions 64:128: PE-shifts the softmax-sum
        # reciprocals from partitions 64:128 down to 0:64 (no DMA round-trip)
        eye_sb = cpool.tile([128, 64], BF16, name="eye_sb")
        nc.sync.dma_start(eye_sb[:], eye[:])

        if WARMUP:
            # ramp the PE p-state (full speed needs ~3us of continuous
            # execution) on junk data while the input DMAs land, so the
            # first projections run at 2.4GHz
            junk = cpool.tile([64, 64], BF16, name="junk")
            nc.gpsimd.memset(junk[:], 0.5)
            jp = apool.tile([64, 64], F32, tag="acc", name="jp")
            for _ in range(WARMUP):
                nc.tensor.matmul(jp[:], junk[:], junk[:],
                                 start=True, stop=True)

        # ---- persistent SBUF tensors ----
        kt_sb = [cpool.tile([128, N], BF16, name=f"kt{h}") for h in range(H)]
        qt_sb = [cpool.tile([128, RPC], BF16, name=f"qt{h}") for h in range(H)]
        PVDT = BF16 if pv == "bf16" else F8
        v_sb = cpool.tile([128, NKV, H, 128], PVDT, name="v_sb")
        # ones columns for the PV denominator trick: memset on the (idle)
        # Pool engine so DVE is free for projection copies at startup
        nc.gpsimd.memset(v_sb[:, :, :, 64:128], 1.0)
        o_sb = [cpool.tile([64, RPC], BF16, name=f"o{h}") for h in range(H)]
        micro = mode in ("qkonly", "pvonly")
        nu_ = H * NQT
        if micro or mode == "nopv" or (
                num_units is not None and num_units < nu_):
            # partial-unit timing variants read o_sb regions no unit wrote
            for h in range(H):
                nc.gpsimd.memset(o_sb[h][:], 0.0)
        if micro:
            for h in range(H):
                nc.vector.memset(kt_sb[h][:], 0.125)
                nc.vector.memset(qt_sb[h][:], 0.125)
            nc.vector.memset(v_sb[:], 0.125)
        pt_shared = None
        if mode == "pvonly":
            pt_shared = cpool.tile([128, NKV * QT], PVDT, name="pt_shared")
            nc.vector.memset(pt_shared[:], 0.125)

        def kq_tile(h, kind, j, tag="acc", pool=None):
            """One K^T or Q^T projection tile (512 cols) for head h."""
            w_sb, dst = (wk_sb, kt_sb) if kind == "k" else (wq_sb, qt_sb)
            ps = (pool or apool).tile([128, 512], F32, tag=tag, name=f"{kind}proj")
            for cc in range(2):
                nc.tensor.matmul(
                    ps[:],
                    w_sb[cc][:, h * 128:(h + 1) * 128],
                    xT_sb[cc][:, j * 512:(j + 1) * 512],
                    start=(cc == 0), stop=(cc == 1),
                )
            nc.vector.tensor_copy(dst[h][:, j * 512:(j + 1) * 512], ps[:])

        def kq_proj_work(h):
            return [
                (900, (lambda kind=kind, j=j: kq_tile(h, kind, j)))
                for kind, nj in (("k", N // 512), ("q", RPC // 512))
                for j in range(nj)
            ]

        def v_pair(j2):
            """V projection for kv chunks 2*j2, 2*j2+1 (one PSUM tile)."""
            ps = apool.tile([128, 512], F32, tag="acc", name="vproj")
            for u in range(2):
                j = 2 * j2 + u
                for cc in range(2):
                    nc.tensor.matmul(
                        ps[:, u * C:(u + 1) * C],
                        xT_sb[cc][:, j * 128:(j + 1) * 128],
                        wv_sb[cc][:],
                        start=(cc == 0), stop=(cc == 1),
                    )
            nc.vector.tensor_copy(
                v_sb[:, 2 * j2:2 * j2 + 2, :, 0:64],
                ps[:].rearrange("p (u h d) -> p u h d", u=2, h=H),
            )

        groups = []
        j = 0
        while j < NKV:
            groups.append((j, min(NKV, j + GRP)))
            j += GRP
        NGRP = len(groups)

        ot_state = {}

        def rowtile_final(qt, rt, flush=None):
            # accumulate the qtile's 128-row output blocks in one SBUF
            # staging tile; strided DMAs ship multi-block spans (HWDGE
            # descriptor generation is the tail serializer, so fewer DMAs)
            if flush is None:
                flush = rt == QT // 128 - 1
            r0 = qt * QT + rt * 128
            po = apool.tile([128, C], F32, tag="acc", name="po")
            for h in range(H):
                nc.tensor.matmul(
                    po[:],
                    o_sb[h][:, r0:r0 + 128],
                    wp_sb[h][:],
                    start=(h == 0), stop=(h == H - 1),
                )
            if qt not in ot_state:
                ot_state[qt] = [
                    opool.tile([128, QT // 128, C], F32, tag="ot", name="ot"),
                    0,
                ]
            tile, fs = ot_state[qt]
            nc.vector.tensor_add(tile[:, rt, :], po[:], bp_sb[:])
            if flush:
                nt = rt - fs + 1
                nc.sync.dma_start(
                    out[qt * QT + fs * 128:qt * QT + (rt + 1) * 128, :]
                    .rearrange("(t p) c -> p t c", t=nt),
                    tile[:, fs:rt + 1, :],
                )
                ot_state[qt][1] = rt + 1

        def pv_norm_work(h, qt, pt, sub=False):
            """Closures for unit (h, qt)'s PV burst (4 chunks each), then
            normalize, then (after head 3) the qtile's output projection.
            sub=True (last unit) pipelines normalize + output projection in
            128-row subtiles so the post-exp tail is short. The PSUM acc
            tile is allocated lazily at the first pv4 call so queued work
            from other units can rotate through the pool meanwhile."""
            cell = {}

            def getacc():
                if "a" not in cell:
                    cell["a"] = apool.tile([128, QT], F32, tag="acc",
                                           name="acc")
                return cell["a"]

            def pv4(j0):
                acc = getacc()
                if pv in ("dr", "dri"):
                    # fp8 DoubleRow: two kv chunks (256-deep contraction)
                    # per matmul; pt chunk pair [128, 2*QT] is pair-major.
                    pm = MPM.DoubleRow if pv == "dr" else \
                        MPM.DoubleRowSwInterleave
                    for j2 in range(j0 // 2, j0 // 2 + 2):
                        nc.tensor.matmul(
                            acc[:],
                            v_sb[:, 2 * j2:2 * j2 + 2, h, :],
                            pt[:, 2 * j2 * QT:(2 * j2 + 2) * QT].rearrange(
                                "p (t r) -> p t r", t=2),
                            start=(j2 == 0), stop=(j2 == NKV // 2 - 1),
                            perf_mode=pm,
                        )
                else:
                    for jj in range(j0, j0 + 4):
                        nc.tensor.matmul(
                            acc[:],
                            v_sb[:, jj, h, :],
                            pt[:, jj * QT:jj * QT + QT],
                            start=(jj == 0), stop=(jj == NKV - 1),
                        )

            def norm_noop():
                pass

            def norm(c0=0, w=QT, direct=False):
                # partitions 0:64 = O'^T, 64:128 = sums (replicated).
                acc = getacc()
                if NORM_PE:
                    # Reciprocal on DVE (bf16 out), PE eye-matmul shifts it
                    # to partitions 0:64 in PSUM, then one DVE multiply
                    # reads both PSUM operands and writes o_sb.
                    rcp = npool.tile([128, w], BF16, tag="rcp", name="rcp")
                    with nc.allow_low_precision(
                            reason="softmax reciprocal in bf16 feeds a bf16 "
                                   "multiply; 2e-2 rel-err budget"):
                        nc.vector.reciprocal(rcp[64:128, :],
                                             acc[64:128, c0:c0 + w])
                    sh = apool.tile([64, w], F32, tag="acc", name="sh")
                    nc.tensor.matmul(
                        sh[:], eye_sb[64:128, :], rcp[64:128, :],
                        start=True, stop=True, tile_position=(64, 0),
                    )
                    nc.vector.tensor_mul(
                        o_sb[h][:, qt * QT + c0:qt * QT + c0 + w],
                        acc[0:64, c0:c0 + w], sh[:],
                    )
                    return
                rcp = npool.tile([128, w], F32, tag="rcp", name="rcp")
                nc.vector.reciprocal(rcp[64:128, :], acc[64:128, c0:c0 + w])
                bc = npool.tile([64, w], F32, tag="bc", name="bc")
                nc.sync.dma_start(bc[:], rcp[64:128, :])
                if direct:
                    # tail path: multiply straight out of PSUM (no copy);
                    # holding the acc slot a little longer is free here
                    nc.vector.tensor_mul(
                        o_sb[h][:, qt * QT + c0:qt * QT + c0 + w],
                        acc[0:64, c0:c0 + w], bc[:],
                    )
                    return
                onum = npool.tile([64, w], F32, tag="onum", name="onum")
                nc.vector.tensor_copy(onum[:], acc[0:64, c0:c0 + w])
                nc.vector.tensor_mul(
                    o_sb[h][:, qt * QT + c0:qt * QT + c0 + w], onum[:], bc[:],
                )

            if mode in ("nopv", "qkonly"):
                work = []
            else:
                pvw = 250 if pv in ("dr", "dri") else PVW
                work = [(pvw, (lambda j0=j0: pv4(j0))) for j0 in range(0, NKV, 4)]
                if sub and not micro and h == H - 1:
                    # last unit: pipeline normalize + output projection in
                    # two 256-row halves so the post-exp tail is short; both
                    # norm DMAs issue before the first output projection
                    hw_ = QT // 2
                    work.append((150, lambda: norm(0, hw_, True)))
                    work.append((150, lambda: norm(hw_, hw_, True)))
                    work.append((600, lambda: rowtile_final(qt, 0, False)))
                    work.append((600, lambda: rowtile_final(qt, 1, True)))
                    work.append((600, lambda: rowtile_final(qt, 2, False)))
                    work.append((600, lambda: rowtile_final(qt, 3, True)))
                    return work
                work.append((150, norm_noop if micro else norm))
            if h == H - 1:
                work.extend(
                    (600, (lambda rt=rt: rowtile_final(qt, rt)))
                    for rt in range(QT // 128)
                )
            return work

        def emit_unit_fifo(h, qt, queue, bcell, drain):
            """QK^T + exp stream for unit (h, qt) under the shared FIFO work
            queue: after each exp group, this unit's own PV entries whose pt
            chunks are now produced are appended, then queued closures run
            under the PE-slack token bucket. No drain at unit end (backlog
            carries across units) unless drain=True (last unit)."""
            pt = ppool.tile([128, NKV * QT], PVDT, tag="pt", name="pt")
            own = list(pv_norm_work(h, qt, pt, sub=drain))
            npv = NKV // 4 if own else 0
            for gi, (j0, j1) in enumerate(groups):
                w = (j1 - j0) * QT
                st = spool.tile([128, GRP * QT], F32, tag="st", name="st")
                for jj in range(j0, j1):
                    rg = (jj % 2) * 64
                    nc.tensor.matmul(
                        st[:, (jj - j0) * QT:(jj - j0) * QT + QT],
                        kt_sb[h][rg:rg + 64, jj * KVC:(jj + 1) * KVC],
                        qt_sb[h][rg:rg + 64, qt * QT:qt * QT + QT],
                        start=True, stop=True,
                        tile_position=(rg, 0),
                    )
                if mode == "noact":
                    nc.scalar.activation(
                        pt[:, j0 * QT:j0 * QT + 64], st[:, 0:64], AF.Exp,
                        scale=SCALE,
                    )
                else:
                    nc.scalar.activation(
                        pt[:, j0 * QT:j1 * QT], st[:, 0:w], AF.Exp, scale=SCALE,
                    )
                # this unit's pv4(j0=4k) is runnable once chunks j0..j0+3
                # are exp'd (group floor((j0+3)/GRP)); the trailing
                # norm/output closures follow the last pv4 in FIFO order
                while npv and 4 * (NKV // 4 - npv) + 3 <= GRP * gi + 2:
                    queue.append(own.pop(0))
                    npv -= 1
                    if npv == 0:
                        queue.extend(own)
                        del own[:]
                budget = min(bcell[0], BURST_NS) + GROUP_NS
                while queue and queue[0][0] <= budget:
                    c, fn = queue.pop(0)
                    fn()
                    budget -= c
                bcell[0] = budget
            if own:
                queue.extend(own)
                del own[:]
            if drain:
                for c, fn in queue:
                    fn()
                del queue[:]
            return pt

        def emit_unit(h, qt, pvq, extra, st_extra=None, self_pv=False,
                      own_push=None):
            """QK^T + exp stream for unit (h, qt); interleaves the previous
            unit's PV/norm closures (pvq), projection closures (extra: [acc]
            slots; st_extra: [st] slots), and optionally its own PV at a
            3-group lag (last unit). own_push (fifo mode): callback(gi) that
            appends this unit's own PV work to the shared queue as the exp
            groups that produce its inputs retire."""
            if mode == "pvonly":
                for c, fn in pvq:
                    fn()
                del pvq[:]
                return pt_shared
            pt = ppool.tile([128, NKV * QT], PVDT, tag="pt", name="pt")
            own = pv_norm_work(h, qt, pt) if self_pv else None
            budget = 0
            for gi, (j0, j1) in enumerate(groups):
                w = (j1 - j0) * QT
                st = spool.tile([128, GRP * QT], F32, tag="st", name="st")
                qw = 64 if mode == "smallqk" else QT
                for jj in range(j0, j1):
                    rg = 0 if mode == "qkflat" else (jj % 2) * 64
                    nc.tensor.matmul(
                        st[:, (jj - j0) * QT:(jj - j0) * QT + qw],
                        kt_sb[h][rg:rg + 64, jj * KVC:(jj + 1) * KVC],
                        qt_sb[h][rg:rg + 64, qt * QT:qt * QT + qw],
                        start=True, stop=True,
                        tile_position=None if mode == "qkflat" else (rg, 0),
                    )
                if mode == "qkonly":
                    pass
                elif mode == "noact":
                    nc.scalar.activation(
                        pt[:, j0 * QT:j0 * QT + 64], st[:, 0:64], AF.Exp,
                        scale=SCALE,
                    )
                else:
                    nc.scalar.activation(
                        pt[:, j0 * QT:j1 * QT], st[:, 0:w], AF.Exp, scale=SCALE,
                    )
                if st_extra:
                    st_extra.pop(0)()
                if own_push is not None:
                    own_push(gi)
                budget = min(budget, BURST_NS) + GROUP_NS
                while pvq and pvq[0][0] <= budget:
                    c, fn = pvq.pop(0)
                    fn()
                    budget -= c
                if own and gi >= 3:
                    own.pop(0)[1]()
            for c, fn in pvq:
                fn()
            del pvq[:]
            if own:
                for c, fn in own:
                    fn()
            return pt

        # ---- pipeline ----
        # Only the first K and Q projection tiles of head 0 run up front (the
        # minimum for unit 0's first QK^T group); the rest of head 0's tiles
        # ride unit 0's [st] rotation (each tile j lands before the group
        # that needs it). V is interleaved into units 0-1 through the [acc]
        # slots (free there since each unit's PV runs inside the NEXT unit);
        # head h+1's K/Q projections spread over head h's units.
        if not micro:
            if mode in ("fifo", "noact"):
                kq_tile(0, "q", 0)
                kq_tile(0, "k", 0)
            else:
                kq_tile(0, "q", 0, tag="st", pool=spool)
                kq_tile(0, "k", 0, tag="st", pool=spool)
        st0 = [] if micro else [
            (lambda j=j: kq_tile(0, "k", j, tag="st", pool=spool))
            for j in range(1, N // 512)
        ] + [
            (lambda j=j: kq_tile(0, "q", j, tag="st", pool=spool))
            for j in range(1, RPC // 512)
        ]

        units = [(h, qt) for h in range(H) for qt in range(NQT)]
        extras = {i: [] for i in range(len(units))}
        vps = [] if micro else [
            (900, (lambda j2=j2: v_pair(j2))) for j2 in range(NKV // 2)
        ]
        extras[0].extend(vps[:8])
        extras[1].extend(vps[8:])
        for nh in range(H if micro else 1, H):
            w = kq_proj_work(nh)
            for k in range(NQT):
                lo = (len(w) * k) // NQT
                hi = (len(w) * (k + 1)) // NQT
                extras[(nh - 1) * NQT + k].extend(w[lo:hi])

        def interleave(a, b):
            out, la, lb = [], list(a), list(b)
            while la or lb:
                if la:
                    out.append(la.pop(0))
                if lb:
                    out.append(lb.pop(0))
            return out

        if mode in ("fifo", "noact") and not micro:
            # single shared FIFO: head-0 K/Q tiles first (feeding unit 0's
            # own groups just-in-time), then V pairs alternating with
            # heads 1-3 K/Q projections; each unit's PV/norm/output work is
            # appended by emit_unit_fifo as its inputs appear. Pops happen
            # after each exp group under the PE-slack token bucket, so the
            # exp stream is never displaced by more than BURST_NS of PE work.
            queue = []
            for j in range(1, N // 512):
                queue.append((KQW, (lambda j=j: kq_tile(0, "k", j))))
            for j in range(1, RPC // 512):
                queue.append((KQW, (lambda j=j: kq_tile(0, "q", j))))
            rest = []
            for nh in range(1, H):
                rest.extend(
                    (KQW, fn2) for _c, fn2 in kq_proj_work(nh))
            vq = [(VPW, (lambda j2=j2: v_pair(j2)))
                  for j2 in range(NKV // 2)]
            queue.extend(interleave(vq, rest))
            nu = len(units) if num_units is None else num_units
            bcell = [0]
            for i, (h, qt) in enumerate(units[:nu]):
                emit_unit_fifo(h, qt, queue, bcell, drain=(i == nu - 1))
            if nu < len(units):
                for c, fn in queue:
                    fn()
                for qt in range(NQT):
                    for rt in range(QT // 128):
                        rowtile_final(qt, rt)
            return nc

        if mode in ("upfront", "mid"):
            if mode == "upfront":
                for w in st0:
                    w()
                del st0[:]
            for i in range(len(units)):
                for c, fn in extras[i]:
                    fn()
                extras[i] = []

        nu = len(units) if num_units is None else num_units
        pvq = []
        for i, (h, qt) in enumerate(units[:nu]):
            last = i == nu - 1
            if i == 1:
                # unit 1's leftover V pairs must precede unit 0's PV chunks
                q = interleave(extras[i][:8], pvq) + extras[i][8:]
            else:
                q = interleave(pvq, extras[i]) if extras[i] else pvq
            pt = emit_unit(
                h, qt, q, [],
                st_extra=(st0 if i == 0 else None), self_pv=last,
            )
            pvq = [] if last else pv_norm_work(h, qt, pt)
        for c, fn in pvq:
            fn()
        if nu < len(units):
            # timing variants: keep PE/DVE/DMA side work + full output
            # traffic identical, just skip the later attention units
            for i in range(nu, len(units)):
                for c, fn in extras[i]:
                    fn()
            for qt in range(NQT):
                for rt in range(QT // 128):
                    rowtile_final(qt, rt)

    return nc


PV_DEFAULT = "bf16"


def _get_program(num_units=None, repeat=1, mode="upfront", pv=None):
    if pv is None:
        pv = PV_DEFAULT
    key = ("nc", num_units, repeat, mode, pv)
    if key not in _cache:
        nc = _build_program(num_units, repeat, mode, pv)
        if not nc.is_finalized():
            nc.finalize()
        _cache[key] = nc
    return _cache[key]


def _prep_inputs(x, Wq, Wk, Wv, Wp, bpv):
    """Build the 8 per-core input maps (host-side shard prep)."""
    bf16 = ml_dtypes.bfloat16
    x = np.asarray(x, dtype=np.float32)

    def dup_heads(w):
        # W [out=C, in=C] (torch Linear) -> W.T [C_in, C_out]; per head 64
        # output cols duplicated to 128; chunk contraction dim into 2x128.
        wt = np.ascontiguousarray(np.asarray(w, dtype=np.float32).T)  # [C_in, C_out]
        cols = []
        for h in range(H):
            blk = wt[:, h * 64:(h + 1) * 64]
            cols.append(np.concatenate([blk, blk], axis=1))  # [C, 128]
        arr = np.concatenate(cols, axis=1)                   # [C, H*128]
        return np.ascontiguousarray(arr.reshape(2, 128, H * 128)).astype(bf16)

    wq_a = dup_heads(Wq)
    wk_a = dup_heads(Wk)
    wv_a = np.ascontiguousarray(
        np.asarray(Wv, dtype=np.float32).T.reshape(2, 128, C)
    ).astype(bf16)
    wp_a = np.ascontiguousarray(
        np.asarray(Wp, dtype=np.float32).T.reshape(H, 64, C)
    ).astype(bf16)
    bp_a = np.ascontiguousarray(
        np.broadcast_to(np.asarray(bpv, dtype=np.float32).reshape(1, C), (128, C))
    )

    eye_a = np.zeros((128, 64), dtype=np.float32)
    eye_a[64:128, :] = np.eye(64, dtype=np.float32)
    eye_a = eye_a.astype(bf16)

    in_maps = []
    for c in range(NCORES):
        b, half = c // 2, c % 2
        xt = np.ascontiguousarray(x[b].T)  # [C, N]
        if half == 1:
            xt = np.concatenate([xt[:, RPC:], xt[:, :RPC]], axis=1)
        in_maps.append({
            "xT": np.ascontiguousarray(xt.reshape(2, 128, N)).astype(bf16),
            "wq": wq_a, "wk": wk_a, "wv": wv_a, "wp": wp_a, "bp": bp_a,
            "eye": eye_a,
        })
    return in_maps


def run_cores(in_maps, trace=False):
    from concourse.bass_utils import run_bass_kernel_spmd
    nc = _get_program()
    return run_bass_kernel_spmd(
        nc, in_maps, list(range(NCORES)), trace=trace,
    )


def kernel(x, Wq, Wk, Wv, Wp, bp):
    in_maps = _prep_inputs(x, Wq, Wk, Wv, Wp, bp)
    res = run_cores(in_maps, trace=bool(int(os.environ.get("KERNEL_TRACE", "0"))))
    full = np.empty((B, N, C), dtype=np.float32)
    for c in range(NCORES):
        b, half = c // 2, c % 2
        full[b, half * RPC:(half + 1) * RPC, :] = res.results[c]["out"]
    _cache["last_results"] = res
    return full

